# revision 1
# baseline (speedup 1.0000x reference)
"""2-layer GAT (PyG-style GATConv x2 + log_softmax) on 8 Trainium2 NeuronCores.

v3: per-edge source rows fetched with batched dma_gather (custom SWDGE ucode,
0.34ns/descriptor vs ~1us per 128-row indirect DMA):
- tables padded to 256B rows (dma_gather element granularity);
- int16 indices: tables split at 32768 rows into low/high halves, each block's
  edge slots grouped [low | high] with chunk-aligned padding;
- gather pieces capped at 1024 indices (ucode breaks above ~1k in this env),
  spread round-robin over 4 SWDGE queues (parallel Q7 core pairs);
- dst attention term via one-hot-transpose matmuls; the transposed one-hot is
  built by a partition-broadcast DMA of the precomputed dst row + one f16
  compare;
- node-phase writes batched 8 blocks/DMA; epilogues batched across blocks;
  single Ln; single out DMA with host unpermute.
"""
import sys
sys.path.insert(0, '/opt/trn_rl_repo')
if '/root/.axon_site' not in sys.path:
    sys.path.insert(0, '/root/.axon_site')

import math
import numpy as np

import concourse.bass as bass
import concourse.bacc as bacc
import concourse.tile as tile
from concourse import mybir
from concourse import bass_utils
from concourse import library_config

F16 = mybir.dt.float16
F32 = mybir.dt.float32
I32 = mybir.dt.int32
I16 = mybir.dt.int16
AX = mybir.AxisListType
ALU = mybir.AluOpType
ACTF = mybir.ActivationFunctionType

CORES = 8
P = 128
XC = 8           # node blocks per staging flush
ROWW = 128       # table row width (f16) = 256B, dma_gather granularity
SPLIT = 32768    # int16 table split
MAXI = 1024      # max indices per dma_gather piece


def _ap(t, off, dims):
    base = t[:]
    return bass.AP(base.tensor, base.offset + off, [list(base.ap[0])] + [list(d) for d in dims])


def _build_program(N, F, H, C, NC, TB1, TB2, LK1, LK2, NTAB, NBLK, NODE_BLKS,
                   ESHIFT, HAS_B1):
    HC = H * C
    OWNPAD = NBLK * P
    NT1 = int(sum(TB1))
    NT2 = int(sum(TB2))
    G1W = 8 + HC + 8        # 80 used cols of a g1 row [as1 | h | ad1]
    G2W = 1 + NC + 1 + 6    # 48 used cols of a g2 row [as2 | h2 | ad2 | pad]
    G2R = 1 + NC
    NOB = NODE_BLKS
    OSB = 56
    GFULL = CORES * OWNPAD

    nc = bacc.Bacc("TRN2", target_bir_lowering=False, debug=False,
                   num_devices=CORES, num_swdge_queues=4)

    xT = nc.dram_tensor("xT", [F, NTAB], F16, kind="ExternalInput").ap()
    w1aug = nc.dram_tensor("w1aug", [F, G1W], F16, kind="ExternalInput").ap()
    b1aug = nc.dram_tensor("b1aug", [1, G1W], F16, kind="ExternalInput").ap()
    w2aug = nc.dram_tensor("w2aug", [HC + 1, G2W], F16, kind="ExternalInput").ap()
    ones1 = nc.dram_tensor("ones1", [1, P], F16, kind="ExternalInput").ap()
    iotaf = nc.dram_tensor("iotaf", [P, P], F16, kind="ExternalInput").ap()
    iotac = nc.dram_tensor("iotac", [P, 1], F16, kind="ExternalInput").ap()
    ident = nc.dram_tensor("ident", [P, P], F16, kind="ExternalInput").ap()
    sg1 = nc.dram_tensor("sg1", [P, NT1 * 8], I16, kind="ExternalInput").ap()
    sg2 = nc.dram_tensor("sg2", [P, NT2 * 8], I16, kind="ExternalInput").ap()
    dstc1 = nc.dram_tensor("dstc1", [P, NT1], F16, kind="ExternalInput").ap()
    dstc2 = nc.dram_tensor("dstc2", [P, NT2], F16, kind="ExternalInput").ap()
    dstrow1 = nc.dram_tensor("dstrow1", [1, NT1 * P], F16, kind="ExternalInput").ap()
    dstrow2 = nc.dram_tensor("dstrow2", [1, NT2 * P], F16, kind="ExternalInput").ap()
    out = nc.dram_tensor("out", [P, NBLK * NC], F16, kind="ExternalOutput").ap()

    with tile.TileContext(nc) as tc:
        with tc.tile_pool(name="const", bufs=1) as cp, \
             tc.tile_pool(name="xp", bufs=2) as xp, \
             tc.tile_pool(name="fsp", bufs=3) as fsp, \
             tc.tile_pool(name="gp", bufs=2) as gp, \
             tc.tile_pool(name="ohp", bufs=2) as ohp, \
             tc.tile_pool(name="bcp", bufs=2) as bcp, \
             tc.tile_pool(name="ohtp", bufs=2) as ohtp, \
             tc.tile_pool(name="vp", bufs=2) as vp, \
             tc.tile_pool(name="ep", bufs=2) as ep, \
             tc.tile_pool(name="epi", bufs=1) as epi, \
             tc.tile_pool(name="ltp", bufs=2) as ltp, \
             tc.tile_pool(name="psA", bufs=2, space="PSUM") as psA, \
             tc.tile_pool(name="psG", bufs=1, space="PSUM") as psG, \
             tc.tile_pool(name="psB", bufs=2, space="PSUM") as psB, \
             tc.tile_pool(name="psC", bufs=1, space="PSUM") as psC, \
             tc.tile_pool(name="psT", bufs=1, space="PSUM") as psT, \
             tc.tile_pool(name="dram", bufs=1, space="DRAM") as dp:

            g1tab = dp.tile([NTAB, ROWW], F16)
            g2own = dp.tile([OWNPAD, G2W], F16)
            g2full = dp.tile([GFULL, G2W], F16, addr_space="Shared")
            g2pad = dp.tile([GFULL, ROWW], F16)

            nc.gpsimd.load_library(library_config.mlp)

            # ---- resident constants -------------------------------------
            iota_sb = cp.tile([P, P], F16)
            nc.sync.dma_start(out=iota_sb[:], in_=iotaf[:, :])
            iotac_sb = cp.tile([P, 1], F16)
            nc.sync.dma_start(out=iotac_sb[:], in_=iotac[:, :])
            ident_sb = cp.tile([P, P], F16)
            nc.sync.dma_start(out=ident_sb[:], in_=ident[:, :])
            w1a0 = cp.tile([P, G1W], F16)
            nc.sync.dma_start(out=w1a0[:], in_=w1aug[0:P, :])
            w1a1 = cp.tile([P, G1W], F16)
            nc.sync.dma_start(out=w1a1[:], in_=w1aug[P:2 * P, :])
            b1a = cp.tile([1, G1W], F16)
            nc.sync.dma_start(out=b1a[:], in_=b1aug[:, :])
            w2a = cp.tile([HC + 1, G2W], F16)
            nc.sync.dma_start(out=w2a[:], in_=w2aug[:, :])
            b2row = cp.tile([1, G2W], F16)
            nc.sync.dma_start(out=b2row[:], in_=w2aug[HC:HC + 1, :])
            ones_sb = cp.tile([1, P], F16)
            nc.sync.dma_start(out=ones_sb[:], in_=ones1[:, :])
            sg1_sb = cp.tile([P, NT1 * 8], I16)
            nc.sync.dma_start(out=sg1_sb[:], in_=sg1[:, :])
            sg2_sb = cp.tile([P, NT2 * 8], I16)
            nc.sync.dma_start(out=sg2_sb[:], in_=sg2[:, :])
            dst1_sb = cp.tile([P, NT1], F16)
            nc.sync.dma_start(out=dst1_sb[:], in_=dstc1[:, :])
            dst2_sb = cp.tile([P, NT2], F16)
            nc.sync.dma_start(out=dst2_sb[:], in_=dstc2[:, :])
            zcol = cp.tile([P, 1], F32)
            nc.vector.memset(zcol[:], 0.0)
            scol = cp.tile([P, 1], F32)
            nc.vector.memset(scol[:], ESHIFT)

            own_stage = cp.tile([P, OSB * G1W], F16)
            g2stage = cp.tile([P, NBLK * G2W], F16)
            aggst1 = cp.tile([P, NBLK * (8 + HC)], F16)
            aggst2 = cp.tile([P, NBLK * G2R], F16)
            smst = cp.tile([P, NBLK], F32)
            h1eal = cp.tile([P, NBLK * HC], F16)

            qctr = [0]

            def gathers(g1c, tab_lo, tab_hi, idx_sb, base, c0, c1):
                """Gather chunks [c0, c1) of a block in <=MAXI pieces."""
                c = c0
                while c < c1:
                    ce = min(c1, c + MAXI // P)
                    n = (ce - c) * P
                    nc.gpsimd.dma_gather(
                        _ap(g1c, c * ROWW, [[ROWW, ce - c], [1, ROWW]]),
                        tab_lo if tab_hi is None else tab_hi,
                        idx_sb[:, (base + c) * 8:(base + ce) * 8],
                        n, n, ROWW, queue_num=qctr[0] % 4)
                    qctr[0] += 1
                    c = ce

            # ---- node phase: g1 table for every node --------------------
            for b in range(NOB):
                g, j = divmod(b, XC)
                nb = min(XC, NOB - g * XC)
                if j == 0:
                    x0 = xp.tile([P, XC * P], F16, tag="x0")
                    nc.sync.dma_start(out=x0[:, 0:nb * P],
                                      in_=xT[0:P, b * P:(b + nb) * P])
                    x1 = xp.tile([P, XC * P], F16, tag="x1")
                    nc.sync.dma_start(out=x1[:, 0:nb * P],
                                      in_=xT[P:2 * P, b * P:(b + nb) * P])
                    if b >= OSB:
                        fst = fsp.tile([P, XC * G1W], F16, tag="fst")
                ps = psA.tile([P, G1W], F32, tag="psA")
                nc.tensor.matmul(out=ps[:], lhsT=x0[:, j * P:(j + 1) * P],
                                 rhs=w1a0[:], start=True, stop=False)
                nc.tensor.matmul(out=ps[:], lhsT=x1[:, j * P:(j + 1) * P],
                                 rhs=w1a1[:], start=False, stop=not HAS_B1)
                if HAS_B1:
                    nc.tensor.matmul(out=ps[:], lhsT=ones_sb[:], rhs=b1a[:],
                                     start=False, stop=True)
                if b < OSB:
                    dst_view = _ap(own_stage, b * G1W, [[1, G1W]])
                else:
                    dst_view = _ap(fst, j * G1W, [[1, G1W]])
                nc.scalar.activation(dst_view, ps[:], ACTF.Copy, bias=0.0)
                if j == nb - 1:
                    src_t = own_stage if b < OSB else fst
                    off0 = g * XC * G1W if b < OSB else 0
                    nc.sync.dma_start(
                        out=g1tab[g * XC * P:g * XC * P + nb * P, 0:G1W],
                        in_=_ap(src_t, off0, [[1, nb * G1W]]))

            g1hi = g1tab[SPLIT:NTAB, :]

            # ---- layer 1 edge phase -------------------------------------
            t0 = 0
            for b in range(NBLK):
                tb = int(TB1[b])
                lk = int(LK1[b])
                g1c = gp.tile([P, tb * ROWW], F16, tag="g1c")
                gathers(g1c, g1tab[0:NTAB, :], None, sg1_sb, t0, 0, lk)
                if tb > lk:
                    gathers(g1c, None, g1hi, sg1_sb, t0, lk, tb)
                ohc = ohp.tile([P, tb * P], F16, tag="ohc")
                nc.vector.tensor_tensor(
                    out=_ap(ohc, 0, [[P, tb], [1, P]]),
                    in0=_ap(iota_sb, 0, [[0, tb], [1, P]]),
                    in1=_ap(dst1_sb, t0, [[1, tb], [0, P]]),
                    op=ALU.is_equal)
                bc = bcp.tile([P, tb * P], F16, tag="bc")
                nc.sync.dma_start(
                    out=bc[:],
                    in_=bass.AP(dstrow1.tensor, dstrow1.offset + t0 * P,
                                [[0, P], [1, tb * P]]))
                ohtc = ohtp.tile([P, tb * P], F16, tag="ohtc")
                nc.vector.tensor_tensor(
                    out=ohtc[:],
                    in0=_ap(iotac_sb, 0, [[0, tb], [0, P]]),
                    in1=bc[:],
                    op=ALU.is_equal)
                adps = psB.tile([P, tb * 8], F32, tag="adps")
                for k in range(tb):
                    nc.tensor.matmul(out=adps[:, k * 8:(k + 1) * 8],
                                     lhsT=ohtc[:, k * P:(k + 1) * P],
                                     rhs=_ap(own_stage, b * G1W + 8 + HC, [[1, 8]]),
                                     start=True, stop=True)
                ech = ep.tile([P, tb * 8], F32, tag="ech")
                nc.vector.tensor_tensor(
                    out=_ap(ech, 0, [[8, tb], [1, 8]]),
                    in0=_ap(g1c, 0, [[ROWW, tb], [1, 8]]),
                    in1=_ap(adps, 0, [[8, tb], [1, 8]]),
                    op=ALU.add)
                lrch = ep.tile([P, tb * 8], F32, tag="lrch")
                nc.vector.scalar_tensor_tensor(out=lrch[:], in0=ech[:], scalar=0.2,
                                               in1=ech[:], op0=ALU.mult, op1=ALU.max)
                pch = ep.tile([P, tb * 8], F32, tag="pch")
                nc.scalar.activation(pch[:], lrch[:], ACTF.Exp, bias=scol[:, 0:1])
                vc = vp.tile([P, tb * (8 + HC)], F16, tag="vc")
                nc.vector.tensor_copy(
                    out=_ap(vc, 0, [[8 + HC, tb], [1, 8]]),
                    in_=_ap(pch, 0, [[8, tb], [1, 8]]))
                nc.vector.tensor_tensor(
                    out=_ap(vc, 8, [[8 + HC, tb], [8, H], [1, C]]),
                    in0=_ap(g1c, 8, [[ROWW, tb], [8, H], [1, C]]),
                    in1=_ap(pch, 0, [[8, tb], [1, H], [0, C]]),
                    op=ALU.mult)
                psagg = psG.tile([P, 8 + HC], F32, tag="psagg")
                for k in range(tb):
                    nc.tensor.matmul(out=psagg[:], lhsT=ohc[:, k * P:(k + 1) * P],
                                     rhs=vc[:, k * (8 + HC):(k + 1) * (8 + HC)],
                                     start=(k == 0), stop=(k == tb - 1))
                nc.scalar.activation(
                    _ap(aggst1, b * (8 + HC), [[1, 8 + HC]]), psagg[:],
                    ACTF.Copy, bias=0.0)
                t0 += tb

            # ---- batched layer-1 epilogue -------------------------------
            G1S = 8 + HC
            a8 = epi.tile([P, NBLK * 8], F32, tag="a8")
            nc.vector.tensor_tensor(
                out=a8[:],
                in0=_ap(own_stage, 0, [[G1W, NBLK], [1, 8]]),
                in1=_ap(own_stage, G1S, [[G1W, NBLK], [1, 8]]),
                op=ALU.add)
            nc.vector.scalar_tensor_tensor(out=a8[:], in0=a8[:], scalar=0.2,
                                           in1=a8[:], op0=ALU.mult, op1=ALU.max)
            p8 = epi.tile([P, NBLK * 8], F32, tag="p8")
            nc.scalar.activation(p8[:], a8[:], ACTF.Exp, bias=scol[:, 0:1])
            nc.vector.tensor_tensor(
                out=a8[:], in0=_ap(aggst1, 0, [[G1S, NBLK], [1, 8]]),
                in1=p8[:], op=ALU.add)
            nc.vector.tensor_scalar_add(out=a8[:], in0=a8[:], scalar1=1e-16)
            nc.vector.reciprocal(out=a8[:], in_=a8[:])
            hw1 = epi.tile([P, NBLK * HC], F32, tag="hw1")
            nc.vector.tensor_tensor(
                out=hw1[:],
                in0=_ap(own_stage, 8, [[G1W, NBLK], [8, H], [1, C]]),
                in1=_ap(p8, 0, [[8, NBLK], [1, H], [0, C]]),
                op=ALU.mult)
            nc.vector.tensor_tensor(
                out=hw1[:], in0=_ap(aggst1, 8, [[G1S, NBLK], [8, H], [1, C]]),
                in1=hw1[:], op=ALU.add)
            nc.vector.tensor_tensor(
                out=hw1[:], in0=hw1[:],
                in1=_ap(a8, 0, [[8, NBLK], [1, H], [0, C]]),
                op=ALU.mult)
            hw2s = epi.tile([P, NBLK * HC], F32, tag="hw2s")
            nc.vector.tensor_scalar_min(out=hw2s[:], in0=hw1[:], scalar1=0.0)
            nc.scalar.activation(hw2s[:], hw2s[:], ACTF.Exp, bias=zcol[:, 0:1])
            nc.vector.tensor_scalar_max(out=hw1[:], in0=hw1[:], scalar1=0.0)
            nc.vector.tensor_tensor(out=hw1[:], in0=hw2s[:], in1=hw1[:], op=ALU.add)
            nc.vector.tensor_scalar_add(out=h1eal[:], in0=hw1[:], scalar1=-1.0)
            for b in range(NBLK):
                trp = psT.tile([HC, P], F16, tag="trp")
                nc.tensor.transpose(out=trp[:], in_=h1eal[:, b * HC:(b + 1) * HC],
                                    identity=ident_sb[:])
                lt = ltp.tile([HC, P], F16, tag="lt")
                nc.scalar.activation(lt[:], trp[:], ACTF.Copy, bias=0.0)
                ps2 = psC.tile([P, G2W], F32, tag="ps2")
                nc.tensor.matmul(out=ps2[:], lhsT=lt[:], rhs=w2a[0:HC, :],
                                 start=True, stop=False)
                nc.tensor.matmul(out=ps2[:], lhsT=ones_sb[:], rhs=b2row[:],
                                 start=False, stop=True)
                nc.scalar.activation(
                    _ap(g2stage, b * G2W, [[1, G2W]]), ps2[:],
                    ACTF.Copy, bias=0.0)
            nc.sync.dma_start(out=g2own[0:OWNPAD, :], in_=g2stage[:])

            # ---- halo exchange + 256B re-pad ----------------------------
            nc.gpsimd.collective_compute(
                "AllGather", ALU.bypass,
                ins=[g2own[:].opt()], outs=[g2full[:].opt()],
                replica_groups=[list(range(CORES))])
            nc.sync.dma_start(out=g2pad[0:GFULL, 0:G2W], in_=g2full[:, :])

            g2hi = g2pad[SPLIT:GFULL, :]

            # ---- layer 2 edge phase -------------------------------------
            t0 = 0
            for b in range(NBLK):
                tb = int(TB2[b])
                lk = int(LK2[b])
                g2c = gp.tile([P, tb * ROWW], F16, tag="g2c")
                gathers(g2c, g2pad[0:GFULL, :], None, sg2_sb, t0, 0, lk)
                if tb > lk:
                    gathers(g2c, None, g2hi, sg2_sb, t0, lk, tb)
                ohc = ohp.tile([P, tb * P], F16, tag="ohc")
                nc.vector.tensor_tensor(
                    out=_ap(ohc, 0, [[P, tb], [1, P]]),
                    in0=_ap(iota_sb, 0, [[0, tb], [1, P]]),
                    in1=_ap(dst2_sb, t0, [[1, tb], [0, P]]),
                    op=ALU.is_equal)
                bc = bcp.tile([P, tb * P], F16, tag="bc")
                nc.sync.dma_start(
                    out=bc[:],
                    in_=bass.AP(dstrow2.tensor, dstrow2.offset + t0 * P,
                                [[0, P], [1, tb * P]]))
                ohtc = ohtp.tile([P, tb * P], F16, tag="ohtc")
                nc.vector.tensor_tensor(
                    out=ohtc[:],
                    in0=_ap(iotac_sb, 0, [[0, tb], [0, P]]),
                    in1=bc[:],
                    op=ALU.is_equal)
                adps = psB.tile([P, tb], F32, tag="adps")
                for k in range(tb):
                    nc.tensor.matmul(out=adps[:, k:k + 1],
                                     lhsT=ohtc[:, k * P:(k + 1) * P],
                                     rhs=_ap(g2stage, b * G2W + G2R, [[1, 1]]),
                                     start=True, stop=True)
                ech = ep.tile([P, tb], F32, tag="ech2")
                nc.vector.tensor_tensor(
                    out=ech[:],
                    in0=_ap(g2c, 0, [[ROWW, tb]]),
                    in1=adps[:],
                    op=ALU.add)
                lrch = ep.tile([P, tb], F32, tag="lrch2")
                nc.vector.scalar_tensor_tensor(out=lrch[:], in0=ech[:], scalar=0.2,
                                               in1=ech[:], op0=ALU.mult, op1=ALU.max)
                pch = ep.tile([P, tb], F32, tag="pch2")
                nc.scalar.activation(pch[:], lrch[:], ACTF.Exp, bias=zcol[:, 0:1])
                vc = vp.tile([P, tb * G2R], F16, tag="vc2")
                nc.vector.tensor_copy(out=_ap(vc, 0, [[G2R, tb]]), in_=pch[:])
                nc.vector.tensor_tensor(
                    out=_ap(vc, 1, [[G2R, tb], [1, NC]]),
                    in0=_ap(g2c, 1, [[ROWW, tb], [1, NC]]),
                    in1=_ap(pch, 0, [[1, tb], [0, NC]]),
                    op=ALU.mult)
                psagg = psG.tile([P, G2R], F32, tag="psagg2")
                for k in range(tb):
                    nc.tensor.matmul(out=psagg[:], lhsT=ohc[:, k * P:(k + 1) * P],
                                     rhs=vc[:, k * G2R:(k + 1) * G2R],
                                     start=(k == 0), stop=(k == tb - 1))
                nc.scalar.activation(
                    _ap(aggst2, b * G2R, [[1, G2R]]), psagg[:],
                    ACTF.Copy, bias=0.0)
                t0 += tb

            # ---- batched layer-2 epilogue + log_softmax -----------------
            b1c = epi.tile([P, NBLK], F32, tag="b1c")
            nc.vector.tensor_tensor(
                out=b1c[:],
                in0=_ap(g2stage, 0, [[G2W, NBLK]]),
                in1=_ap(g2stage, G2R, [[G2W, NBLK]]),
                op=ALU.add)
            nc.vector.scalar_tensor_tensor(out=b1c[:], in0=b1c[:], scalar=0.2,
                                           in1=b1c[:], op0=ALU.mult, op1=ALU.max)
            pb = epi.tile([P, NBLK], F32, tag="pb")
            nc.scalar.activation(pb[:], b1c[:], ACTF.Exp, bias=zcol[:, 0:1])
            nc.vector.tensor_tensor(
                out=b1c[:], in0=_ap(aggst2, 0, [[G2R, NBLK]]),
                in1=pb[:], op=ALU.add)
            nc.vector.tensor_scalar_add(out=b1c[:], in0=b1c[:], scalar1=1e-16)
            nc.vector.reciprocal(out=b1c[:], in_=b1c[:])
            hn1 = epi.tile([P, NBLK * NC], F32, tag="hn1")
            nc.vector.tensor_tensor(
                out=hn1[:],
                in0=_ap(g2stage, 1, [[G2W, NBLK], [1, NC]]),
                in1=_ap(pb, 0, [[1, NBLK], [0, NC]]),
                op=ALU.mult)
            nc.vector.tensor_tensor(
                out=hn1[:], in0=_ap(aggst2, 1, [[G2R, NBLK], [1, NC]]),
                in1=hn1[:], op=ALU.add)
            nc.vector.tensor_tensor(
                out=hn1[:], in0=hn1[:],
                in1=_ap(b1c, 0, [[1, NBLK], [0, NC]]),
                op=ALU.mult)
            m2 = epi.tile([P, NBLK], F32, tag="m2")
            nc.vector.reduce_max(m2[:], _ap(hn1, 0, [[NC, NBLK], [1, NC]]),
                                 axis=AX.X)
            nc.vector.tensor_tensor(
                out=hn1[:], in0=hn1[:],
                in1=_ap(m2, 0, [[1, NBLK], [0, NC]]),
                op=ALU.subtract)
            hn2 = epi.tile([P, NBLK * NC], F16, tag="hn2")
            nc.scalar.activation(hn2[:], hn1[:], ACTF.Exp, bias=zcol[:, 0:1])
            nc.vector.reduce_sum(smst[:], _ap(hn2, 0, [[NC, NBLK], [1, NC]]),
                                 axis=AX.X)
            nc.scalar.activation(m2[:], smst[:], ACTF.Ln, bias=zcol[:, 0:1])
            nc.vector.tensor_tensor(
                out=hn2[:], in0=hn1[:],
                in1=_ap(m2, 0, [[1, NBLK], [0, NC]]),
                op=ALU.subtract)
            nc.sync.dma_start(out=out[:, :], in_=hn2[:])

    nc.compile()
    return nc


def _pos_map(NOB):
    def pos(tn):
        tn = np.asarray(tn)
        b = tn // P
        p = tn % P
        g = b // XC
        j = b % XC
        nb = np.minimum(XC, NOB - g * XC)
        return g * (XC * P) + p * nb + j
    return pos


def _prep(x, edge_src, edge_dst, W1, a1_src, a1_dst, b1, W2, a2_src, a2_dst, b2):
    N, F = x.shape
    H, C = a1_src.shape
    NC = W2.shape[1]
    HC = H * C
    NOWN = N // CORES
    NBLK = math.ceil(NOWN / P)
    OWNPAD = NBLK * P
    NFOR = N - NOWN
    FBLK = math.ceil(NFOR / P)
    NODE_BLKS = NBLK + FBLK
    NTAB = NODE_BLKS * P
    G1W = 8 + HC + 8

    pos = _pos_map(NODE_BLKS)

    core_of = edge_dst // NOWN
    per_core = []
    for c in range(CORES):
        m = core_of == c
        s, d = edge_src[m], edge_dst[m] - c * NOWN
        blk = d // P
        order = np.argsort(blk, kind='stable')
        per_core.append((s[order], d[order], blk[order]))

    # per-core per-block (low, high) edge lists by layer; global chunk counts
    def layer_split(pos_of_src):
        """-> per-core lists of (low_idx_arrays, high_idx_arrays, d arrays)"""
        lows, highs = np.zeros((CORES, NBLK), np.int64), np.zeros((CORES, NBLK), np.int64)
        data = []
        for c in range(CORES):
            s, d, blk = per_core[c]
            ps = pos_of_src(c, s)
            bstart = np.concatenate([[0], np.cumsum(np.bincount(blk, minlength=NBLK))])
            rows = []
            for b in range(NBLK):
                lo, hi = bstart[b], bstart[b + 1]
                pb, db = ps[lo:hi], d[lo:hi]
                il = np.argsort(np.where(pb < SPLIT, pb, 1 << 30), kind='stable')
                nl = int((pb < SPLIT).sum())
                lowo, higho = il[:nl], il[nl:]
                rows.append((pb[lowo], db[lowo], pb[higho] - SPLIT, db[higho]))
                lows[c, b], highs[c, b] = nl, len(pb) - nl
            data.append(rows)
        LK = np.maximum(0, np.ceil(lows.max(axis=0) / P)).astype(np.int64)
        HK = np.maximum(0, np.ceil(highs.max(axis=0) / P)).astype(np.int64)
        LK = np.maximum(LK, 1)      # at least one chunk so the block exists
        TB = LK + HK
        return data, LK, TB

    pos1 = {}
    for c in range(CORES):
        own_lo, own_hi = c * NOWN, (c + 1) * NOWN
        fore = np.concatenate([np.arange(0, own_lo), np.arange(own_hi, N)])
        tn_of = np.empty(N, np.int64)
        tn_of[own_lo:own_hi] = np.arange(NOWN)
        tn_of[fore] = OWNPAD + np.arange(NFOR)
        pos1[c] = (tn_of, fore)

    data1, LK1, TB1 = layer_split(lambda c, s: pos(pos1[c][0][s]))
    data2, LK2, TB2 = layer_split(
        lambda c, s: (s // NOWN) * OWNPAD + ((s % NOWN) % P) * NBLK + (s % NOWN) // P)
    NT1, NT2 = int(TB1.sum()), int(TB2.sum())
    toff1 = np.concatenate([[0], np.cumsum(TB1)])
    toff2 = np.concatenate([[0], np.cumsum(TB2)])

    W1r = W1.reshape(F, H, C)
    wsrc = (W1r * a1_src[None]).sum(-1)
    wdst = (W1r * a1_dst[None]).sum(-1)
    w1aug = np.concatenate([wsrc, W1, wdst], axis=1).astype(np.float16)
    b1aug = np.zeros((1, G1W), np.float16)
    b1aug[0, 8:8 + HC] = b1.astype(np.float16)
    G2W = 1 + NC + 1 + 6
    w2aug = np.zeros((HC + 1, G2W), np.float16)
    w2aug[0:HC, 0] = (W2 @ a2_src[0]).astype(np.float16)
    w2aug[0:HC, 1:1 + NC] = W2.astype(np.float16)
    w2aug[0:HC, 1 + NC] = (W2 @ a2_dst[0]).astype(np.float16)
    w2aug[HC, 1:1 + NC] = b2.astype(np.float16)
    ones1 = np.ones((1, P), np.float16)
    iotaf = np.tile(np.arange(P, dtype=np.float16)[None, :], (P, 1))
    iotac = np.arange(P, dtype=np.float16)[:, None]
    ident = np.eye(P, dtype=np.float16)

    xT = np.ascontiguousarray(x.T)

    def pack_layer(rows, LK, TB, toff, NT):
        sg = np.zeros((P, NT * 8), np.int16)
        dstc = np.full((P, NT), -1.0, np.float16)
        for b in range(NBLK):
            base = int(toff[b])
            for (vals, dv, coff) in ((rows[b][0], rows[b][1], 0),
                                     (rows[b][2], rows[b][3], int(LK[b]))):
                n = len(vals)
                if n == 0:
                    continue
                i = np.arange(n)
                sg[i % 16, (base + coff) * 8 + i // 16] = vals.astype(np.int16)
                dstc[i % P, base + coff + i // P] = (dv % P).astype(np.float16)
        for rep in range(1, 8):
            sg[rep * 16:(rep + 1) * 16, :] = sg[0:16, :]
        return sg, dstc

    in_maps = []
    for c in range(CORES):
        own_lo, own_hi = c * NOWN, (c + 1) * NOWN
        fore = pos1[c][1]
        xTp = np.zeros((F, NTAB), np.float16)
        xTp[:, 0:NOWN] = xT[:, own_lo:own_hi].astype(np.float16)
        xTp[:, OWNPAD:OWNPAD + NFOR] = xT[:, fore].astype(np.float16)

        sg1, dstc1 = pack_layer(data1[c], LK1, TB1, toff1, NT1)
        sg2, dstc2 = pack_layer(data2[c], LK2, TB2, toff2, NT2)
        in_maps.append({
            "xT": xTp, "w1aug": w1aug, "b1aug": b1aug, "w2aug": w2aug,
            "ones1": ones1, "iotaf": iotaf, "iotac": iotac, "ident": ident,
            "sg1": sg1, "sg2": sg2, "dstc1": dstc1, "dstc2": dstc2,
            "dstrow1": dstc1.T.reshape(1, NT1 * P),
            "dstrow2": dstc2.T.reshape(1, NT2 * P),
        })
    meta = dict(N=N, F=F, H=H, C=C, NC=NC, TB1=TB1, TB2=TB2, LK1=LK1, LK2=LK2,
                NTAB=NTAB, NBLK=NBLK, NODE_BLKS=NODE_BLKS, NOWN=NOWN)
    return in_maps, meta


_CACHED = {}


def run(inputs, eshift=-4.0, trace=False, tmpdir=None):
    in_maps, meta = _prep(**inputs)
    has_b1 = bool(np.any(np.asarray(inputs["b1"])))
    key = (meta["N"], meta["F"], meta["NC"], tuple(meta["TB1"]), tuple(meta["TB2"]),
           has_b1)
    if key not in _CACHED:
        _CACHED[key] = _build_program(meta["N"], meta["F"], meta["H"], meta["C"],
                                      meta["NC"], meta["TB1"], meta["TB2"],
                                      meta["LK1"], meta["LK2"], meta["NTAB"],
                                      meta["NBLK"], meta["NODE_BLKS"], eshift,
                                      has_b1)
    nc = _CACHED[key]
    kw = {"tmpdir": tmpdir} if tmpdir else {}
    res = bass_utils.run_bass_kernel_spmd(nc, in_maps,
                                          core_ids=list(range(CORES)),
                                          trace=trace, **kw)
    NOWN, NBLK, NC = meta["NOWN"], meta["NBLK"], meta["NC"]
    outs = []
    for c in range(CORES):
        o = res.results[c]["out"]
        full = o.reshape(P, NBLK, NC).transpose(1, 0, 2).reshape(NBLK * P, NC)
        outs.append(full[:NOWN])
    full = np.concatenate(outs, axis=0).astype(np.float32)
    return full, res


def kernel(**inputs):
    full, _ = run(inputs)
    return full



# revision 36
# speedup vs baseline: 1.2497x; 1.2497x over previous
"""2-layer GAT (PyG-style GATConv x2 + log_softmax) on 8 Trainium2 NeuronCores.

v3: per-edge source rows fetched with batched dma_gather (custom SWDGE ucode,
0.34ns/descriptor vs ~1us per 128-row indirect DMA):
- tables padded to 256B rows (dma_gather element granularity);
- int16 indices: tables split at 32768 rows into low/high halves, each block's
  edge slots grouped [low | high] with chunk-aligned padding;
- gather pieces capped at 1024 indices (ucode breaks above ~1k in this env),
  spread round-robin over 4 SWDGE queues (parallel Q7 core pairs);
- dst attention term via one-hot-transpose matmuls; the transposed one-hot is
  built by a partition-broadcast DMA of the precomputed dst row + one f16
  compare;
- node-phase writes batched 8 blocks/DMA; epilogues batched across blocks;
  single Ln; single out DMA with host unpermute.
"""
import sys
sys.path.insert(0, '/opt/trn_rl_repo')
if '/root/.axon_site' not in sys.path:
    sys.path.insert(0, '/root/.axon_site')

import math
import numpy as np

import concourse.bass as bass
import concourse.bacc as bacc
import concourse.tile as tile
from concourse import mybir
from concourse import bass_utils
from concourse import library_config

F16 = mybir.dt.float16
F32 = mybir.dt.float32
I32 = mybir.dt.int32
I16 = mybir.dt.int16
AX = mybir.AxisListType
ALU = mybir.AluOpType
ACTF = mybir.ActivationFunctionType

CORES = 8
P = 128
XC = 8           # node blocks per staging flush
ROWW = 128       # table row width (f16) = 256B, dma_gather granularity
SPLIT = 32768    # int16 table split
MAXI = 1024      # max indices per dma_gather piece (>1k breaks the ucode here)
LOOKAHEAD = 4    # blocks of gather descriptor prefetch (prepare_only)
GBUFS = LOOKAHEAD + 2


def _ap(t, off, dims):
    base = t[:]
    return bass.AP(base.tensor, base.offset + off, [list(base.ap[0])] + [list(d) for d in dims])


def _build_program(N, F, H, C, NC, TB1, TB2, LK1, LK2, NTAB, NBLK, NODE_BLKS,
                   ESHIFT, HAS_B1):
    HC = H * C
    OWNPAD = NBLK * P
    NT1 = int(sum(TB1))
    NT2 = int(sum(TB2))
    G1W = 8 + HC + 8        # 80 used cols of a g1 row [as1 | h | ad1]
    G2W = 1 + NC + 1 + 6    # 48 used cols of a g2 row [as2 | h2 | ad2 | pad]
    G2R = 1 + NC
    NOB = NODE_BLKS
    OSB = 56
    NBLK2 = NBLK + (NBLK % 2)   # pad to even so g2 rows pack in pairs
    GP2 = P * NBLK2 // 2        # packed pair-rows per core
    GFULL2 = CORES * GP2        # packed gather-table rows (< 32768)

    nc = bacc.Bacc("TRN2", target_bir_lowering=False, debug=False,
                   num_devices=CORES, num_swdge_queues=4,
                   dynamic_dma_scratch_size=32768)

    xT = nc.dram_tensor("xT", [F, NTAB], F16, kind="ExternalInput").ap()
    w1aug = nc.dram_tensor("w1aug", [F, G1W], F16, kind="ExternalInput").ap()
    b1aug = nc.dram_tensor("b1aug", [1, G1W], F16, kind="ExternalInput").ap()
    w2aug = nc.dram_tensor("w2aug", [HC + 1, G2W], F16, kind="ExternalInput").ap()
    ones1 = nc.dram_tensor("ones1", [1, P], F16, kind="ExternalInput").ap()
    iotaf = nc.dram_tensor("iotaf", [P, P], F16, kind="ExternalInput").ap()
    iotac = nc.dram_tensor("iotac", [P, 1], F16, kind="ExternalInput").ap()
    ident = nc.dram_tensor("ident", [P, P], F16, kind="ExternalInput").ap()
    sg1 = nc.dram_tensor("sg1", [P, NT1 * 8], I16, kind="ExternalInput").ap()
    sg2 = nc.dram_tensor("sg2", [P, NT2 * 8], I16, kind="ExternalInput").ap()
    dstc1 = nc.dram_tensor("dstc1", [P, NT1], F16, kind="ExternalInput").ap()
    dstc2 = nc.dram_tensor("dstc2", [P, NT2], F16, kind="ExternalInput").ap()
    dstrow1 = nc.dram_tensor("dstrow1", [1, NT1 * P], F16, kind="ExternalInput").ap()
    dstrow2 = nc.dram_tensor("dstrow2", [1, NT2 * P], F16, kind="ExternalInput").ap()
    out = nc.dram_tensor("out", [P, NBLK * NC], F16, kind="ExternalOutput").ap()

    with tile.TileContext(nc) as tc:
        with tc.tile_pool(name="const", bufs=1) as cp, \
             tc.tile_pool(name="xp", bufs=2) as xp, \
             tc.tile_pool(name="fsp", bufs=3) as fsp, \
             tc.tile_pool(name="gp", bufs=GBUFS) as gp, \
             tc.tile_pool(name="ohp", bufs=2) as ohp, \
             tc.tile_pool(name="bcp", bufs=2) as bcp, \
             tc.tile_pool(name="ohtp", bufs=2) as ohtp, \
             tc.tile_pool(name="vp", bufs=2) as vp, \
             tc.tile_pool(name="ep", bufs=2) as ep, \
             tc.tile_pool(name="epi", bufs=1) as epi, \
             tc.tile_pool(name="ltp", bufs=2) as ltp, \
             tc.tile_pool(name="psA", bufs=2, space="PSUM") as psA, \
             tc.tile_pool(name="psG", bufs=1, space="PSUM") as psG, \
             tc.tile_pool(name="psB", bufs=2, space="PSUM") as psB, \
             tc.tile_pool(name="psC", bufs=1, space="PSUM") as psC, \
             tc.tile_pool(name="psT", bufs=1, space="PSUM") as psT, \
             tc.tile_pool(name="dram", bufs=1, space="DRAM") as dp:

            g1tab = dp.tile([NTAB, ROWW], F16)
            g2packown = dp.tile([GP2, ROWW], F16)
            g2packfull = dp.tile([GFULL2, ROWW], F16, addr_space="Shared")

            nc.gpsimd.load_library(library_config.mlp)

            # ---- resident constants -------------------------------------
            iota_sb = cp.tile([P, P], F16)
            nc.sync.dma_start(out=iota_sb[:], in_=iotaf[:, :])
            iotac_sb = cp.tile([P, 1], F16)
            nc.sync.dma_start(out=iotac_sb[:], in_=iotac[:, :])
            ident_sb = cp.tile([P, P], F16)
            nc.sync.dma_start(out=ident_sb[:], in_=ident[:, :])
            w1a0 = cp.tile([P, G1W], F16)
            nc.sync.dma_start(out=w1a0[:], in_=w1aug[0:P, :])
            w1a1 = cp.tile([P, G1W], F16)
            nc.sync.dma_start(out=w1a1[:], in_=w1aug[P:2 * P, :])
            b1a = cp.tile([1, G1W], F16)
            nc.sync.dma_start(out=b1a[:], in_=b1aug[:, :])
            w2a = cp.tile([HC + 1, G2W], F16)
            nc.sync.dma_start(out=w2a[:], in_=w2aug[:, :])
            b2row = cp.tile([1, G2W], F16)
            nc.sync.dma_start(out=b2row[:], in_=w2aug[HC:HC + 1, :])
            ones_sb = cp.tile([1, P], F16)
            nc.sync.dma_start(out=ones_sb[:], in_=ones1[:, :])
            sg1_sb = cp.tile([P, NT1 * 8], I16)
            nc.sync.dma_start(out=sg1_sb[:], in_=sg1[:, :])
            sg2_sb = cp.tile([P, NT2 * 8], I16)
            nc.sync.dma_start(out=sg2_sb[:], in_=sg2[:, :])
            dst1_sb = cp.tile([P, NT1], F16)
            nc.sync.dma_start(out=dst1_sb[:], in_=dstc1[:, :])
            dst2_sb = cp.tile([P, NT2], F16)
            nc.sync.dma_start(out=dst2_sb[:], in_=dstc2[:, :])
            zcol = cp.tile([P, 1], F32)
            nc.vector.memset(zcol[:], 0.0)
            scol = cp.tile([P, 1], F32)
            nc.vector.memset(scol[:], ESHIFT)

            own_stage = cp.tile([P, OSB * G1W], F16)
            g2stage = cp.tile([P, NBLK2 * G2W], F16)
            aggst1 = cp.tile([P, NBLK * (8 + HC)], F16)
            aggst2 = cp.tile([P, NBLK * G2R], F16)
            smst = cp.tile([P, NBLK], F32)
            h1eal = cp.tile([P, NBLK * HC], F16)

            # ---- SWDGE gather pipeline: gathers for block b+LOOKAHEAD are
            # emitted before block b's compute, so the Q7 desc-gen and the
            # gather DMAs run several blocks ahead of the consumers (bounded
            # by the gather-tile pool depth and the descriptor ring).
            TBMAX = max(int(max(TB1)), int(max(TB2)))

            def plan(TB, LK):
                out = []
                for b in range(NBLK):
                    tb, lk = int(TB[b]), int(LK[b])
                    ps = []
                    for lo, hi in ((0, lk), (lk, tb)):
                        c = lo
                        while c < hi:
                            ce = min(hi, c + MAXI // P)
                            ps.append((c, ce, lo != 0))
                            c = ce
                    out.append(ps)
                return out

            def prep_block(pieces, gtile, tab_lo, tab_hi, idx_sb, t0, q):
                for (c, ce, hi) in pieces:
                    n = (ce - c) * P
                    nc.gpsimd.dma_gather(
                        _ap(gtile, c * ROWW, [[ROWW, ce - c], [1, ROWW]]),
                        tab_hi if hi else tab_lo,
                        idx_sb[:, (t0 + c) * 8:(t0 + ce) * 8],
                        n, n, ROWW, queue_num=q)

            plan1 = plan(TB1, LK1)
            toff1v = [0]
            for b in range(NBLK):
                toff1v.append(toff1v[-1] + int(TB1[b]))
            g1lo = g1tab[0:NTAB, :]
            g1hi = g1tab[SPLIT:NTAB, :]
            tiles1 = {}

            def prep1(b):
                gt = gp.tile([P, TBMAX * ROWW], F16, tag="gc")
                prep_block(plan1[b], gt, g1lo, g1hi, sg1_sb, toff1v[b], b % 4)
                tiles1[b] = gt

            # ---- node phase: g1 table for every node --------------------
            for b in range(NOB):
                g, j = divmod(b, XC)
                nb = min(XC, NOB - g * XC)
                if j == 0:
                    x0 = xp.tile([P, XC * P], F16, tag="x0")
                    nc.sync.dma_start(out=x0[:, 0:nb * P],
                                      in_=xT[0:P, b * P:(b + nb) * P])
                    x1 = xp.tile([P, XC * P], F16, tag="x1")
                    nc.sync.dma_start(out=x1[:, 0:nb * P],
                                      in_=xT[P:2 * P, b * P:(b + nb) * P])
                    if b >= OSB:
                        fst = fsp.tile([P, XC * G1W], F16, tag="fst")
                ps = psA.tile([P, G1W], F32, tag="psA")
                nc.tensor.matmul(out=ps[:], lhsT=x0[:, j * P:(j + 1) * P],
                                 rhs=w1a0[:], start=True, stop=False)
                nc.tensor.matmul(out=ps[:], lhsT=x1[:, j * P:(j + 1) * P],
                                 rhs=w1a1[:], start=False, stop=not HAS_B1)
                if HAS_B1:
                    nc.tensor.matmul(out=ps[:], lhsT=ones_sb[:], rhs=b1a[:],
                                     start=False, stop=True)
                if b < OSB:
                    dst_view = _ap(own_stage, b * G1W, [[1, G1W]])
                else:
                    dst_view = _ap(fst, j * G1W, [[1, G1W]])
                nc.scalar.activation(dst_view, ps[:], ACTF.Copy, bias=0.0)
                if j == nb - 1:
                    src_t = own_stage if b < OSB else fst
                    off0 = g * XC * G1W if b < OSB else 0
                    nc.sync.dma_start(
                        out=g1tab[g * XC * P:g * XC * P + nb * P, 0:G1W],
                        in_=_ap(src_t, off0, [[1, nb * G1W]]))

            for b in range(min(LOOKAHEAD, NBLK)):
                prep1(b)

            # ---- layer 1 edge phase -------------------------------------
            t0 = 0
            for b in range(NBLK):
                tb = int(TB1[b])
                if b + LOOKAHEAD < NBLK:
                    prep1(b + LOOKAHEAD)
                g1c = tiles1.pop(b)
                ohc = ohp.tile([P, tb * P], F16, tag="ohc")
                nc.vector.tensor_tensor(
                    out=_ap(ohc, 0, [[P, tb], [1, P]]),
                    in0=_ap(iota_sb, 0, [[0, tb], [1, P]]),
                    in1=_ap(dst1_sb, t0, [[1, tb], [0, P]]),
                    op=ALU.is_equal)
                bc = bcp.tile([P, tb * P], F16, tag="bc")
                nc.sync.dma_start(
                    out=bc[:],
                    in_=bass.AP(dstrow1.tensor, dstrow1.offset + t0 * P,
                                [[0, P], [1, tb * P]]))
                ohtc = ohtp.tile([P, tb * P], F16, tag="ohtc")
                nc.vector.tensor_tensor(
                    out=ohtc[:],
                    in0=_ap(iotac_sb, 0, [[0, tb], [0, P]]),
                    in1=bc[:],
                    op=ALU.is_equal)
                adps = psB.tile([P, tb * 8], F32, tag="adps")
                for k in range(tb):
                    nc.tensor.matmul(out=adps[:, k * 8:(k + 1) * 8],
                                     lhsT=ohtc[:, k * P:(k + 1) * P],
                                     rhs=_ap(own_stage, b * G1W + 8 + HC, [[1, 8]]),
                                     start=True, stop=True)
                ech = ep.tile([P, tb * 8], F32, tag="ech")
                nc.vector.tensor_tensor(
                    out=_ap(ech, 0, [[8, tb], [1, 8]]),
                    in0=_ap(g1c, 0, [[ROWW, tb], [1, 8]]),
                    in1=_ap(adps, 0, [[8, tb], [1, 8]]),
                    op=ALU.add)
                lrch = ep.tile([P, tb * 8], F32, tag="lrch")
                nc.vector.scalar_tensor_tensor(out=lrch[:], in0=ech[:], scalar=0.2,
                                               in1=ech[:], op0=ALU.mult, op1=ALU.max)
                pch = ep.tile([P, tb * 8], F32, tag="pch")
                nc.scalar.activation(pch[:], lrch[:], ACTF.Exp, bias=scol[:, 0:1])
                vc = vp.tile([P, tb * (8 + HC)], F16, tag="vc")
                nc.vector.tensor_copy(
                    out=_ap(vc, 0, [[8 + HC, tb], [1, 8]]),
                    in_=_ap(pch, 0, [[8, tb], [1, 8]]))
                nc.vector.tensor_tensor(
                    out=_ap(vc, 8, [[8 + HC, tb], [8, H], [1, C]]),
                    in0=_ap(g1c, 8, [[ROWW, tb], [8, H], [1, C]]),
                    in1=_ap(pch, 0, [[8, tb], [1, H], [0, C]]),
                    op=ALU.mult)
                psagg = psG.tile([P, 8 + HC], F32, tag="psagg")
                for k in range(tb):
                    nc.tensor.matmul(out=psagg[:], lhsT=ohc[:, k * P:(k + 1) * P],
                                     rhs=vc[:, k * (8 + HC):(k + 1) * (8 + HC)],
                                     start=(k == 0), stop=(k == tb - 1))
                nc.scalar.activation(
                    _ap(aggst1, b * (8 + HC), [[1, 8 + HC]]), psagg[:],
                    ACTF.Copy, bias=0.0)
                t0 += tb

            # L2 gather descriptors generate on gpsimd while the L1 epilogue
            # + halo exchange run; the DMAs only fire at trigger time, after
            # g2pad is written.
            plan2 = plan(TB2, TB2)  # single packed table: pieces span classes
            toff2v = [0]
            for b in range(NBLK):
                toff2v.append(toff2v[-1] + int(TB2[b]))
            g2lo = g2packfull[0:GFULL2, :]
            tiles2 = {}

            def prep2(b):
                gt = gp.tile([P, TBMAX * ROWW], F16, tag="gc")
                prep_block(plan2[b], gt, g2lo, g2lo, sg2_sb, toff2v[b], b % 4)
                tiles2[b] = gt

            # ---- batched layer-1 epilogue -------------------------------
            G1S = 8 + HC
            a8 = epi.tile([P, NBLK * 8], F32, tag="a8")
            nc.vector.tensor_tensor(
                out=a8[:],
                in0=_ap(own_stage, 0, [[G1W, NBLK], [1, 8]]),
                in1=_ap(own_stage, G1S, [[G1W, NBLK], [1, 8]]),
                op=ALU.add)
            nc.vector.scalar_tensor_tensor(out=a8[:], in0=a8[:], scalar=0.2,
                                           in1=a8[:], op0=ALU.mult, op1=ALU.max)
            p8 = epi.tile([P, NBLK * 8], F32, tag="p8")
            nc.scalar.activation(p8[:], a8[:], ACTF.Exp, bias=scol[:, 0:1])
            nc.vector.tensor_tensor(
                out=a8[:], in0=_ap(aggst1, 0, [[G1S, NBLK], [1, 8]]),
                in1=p8[:], op=ALU.add)
            nc.vector.tensor_scalar_add(out=a8[:], in0=a8[:], scalar1=1e-16)
            nc.vector.reciprocal(out=a8[:], in_=a8[:])
            hw1 = epi.tile([P, NBLK * HC], F32, tag="hw1")
            nc.vector.tensor_tensor(
                out=hw1[:],
                in0=_ap(own_stage, 8, [[G1W, NBLK], [8, H], [1, C]]),
                in1=_ap(p8, 0, [[8, NBLK], [1, H], [0, C]]),
                op=ALU.mult)
            nc.vector.tensor_tensor(
                out=hw1[:], in0=_ap(aggst1, 8, [[G1S, NBLK], [8, H], [1, C]]),
                in1=hw1[:], op=ALU.add)
            nc.vector.tensor_tensor(
                out=hw1[:], in0=hw1[:],
                in1=_ap(a8, 0, [[8, NBLK], [1, H], [0, C]]),
                op=ALU.mult)
            hw2s = epi.tile([P, NBLK * HC], F32, tag="hw2s")
            nc.vector.tensor_scalar_min(out=hw2s[:], in0=hw1[:], scalar1=0.0)
            nc.scalar.activation(hw2s[:], hw2s[:], ACTF.Exp, bias=zcol[:, 0:1])
            nc.vector.tensor_scalar_max(out=hw1[:], in0=hw1[:], scalar1=0.0)
            nc.vector.tensor_tensor(out=hw1[:], in0=hw2s[:], in1=hw1[:], op=ALU.add)
            nc.vector.tensor_scalar_add(out=h1eal[:], in0=hw1[:], scalar1=-1.0)
            for b in range(NBLK):
                trp = psT.tile([HC, P], F16, tag="trp")
                nc.tensor.transpose(out=trp[:], in_=h1eal[:, b * HC:(b + 1) * HC],
                                    identity=ident_sb[:])
                lt = ltp.tile([HC, P], F16, tag="lt")
                nc.scalar.activation(lt[:], trp[:], ACTF.Copy, bias=0.0)
                ps2 = psC.tile([P, G2W], F32, tag="ps2")
                nc.tensor.matmul(out=ps2[:], lhsT=lt[:], rhs=w2a[0:HC, :],
                                 start=True, stop=False)
                nc.tensor.matmul(out=ps2[:], lhsT=ones_sb[:], rhs=b2row[:],
                                 start=False, stop=True)
                nc.scalar.activation(
                    _ap(g2stage, b * G2W, [[1, G2W]]), ps2[:],
                    ACTF.Copy, bias=0.0)
            # pack pairs of g2 rows into 256B table rows: row p*25+j holds
            # [g2(p, 2j) | g2(p, 2j+1) | pad]; the AllGather output is then
            # directly the layer-2 gather table (no re-pad step).
            gpo = g2packown[:]
            nc.sync.dma_start(
                out=bass.AP(gpo.tensor, gpo.offset,
                            [[NBLK2 // 2 * ROWW, P], [ROWW, NBLK2 // 2],
                             [1, 2 * G2W]]),
                in_=_ap(g2stage, 0, [[2 * G2W, NBLK2 // 2], [1, 2 * G2W]]))

            # ---- halo exchange --------------------------------------
            nc.gpsimd.collective_compute(
                "AllGather", ALU.bypass,
                ins=[g2packown[:].opt()], outs=[g2packfull[:].opt()],
                replica_groups=[list(range(CORES))])
            # L2 gathers read the AG output: they MUST be emitted after the
            # collective (same engine queue; Tile gates them on completion)
            for b in range(min(LOOKAHEAD, NBLK)):
                prep2(b)

            # ---- layer 2 edge phase -------------------------------------
            t0 = 0
            for b in range(NBLK):
                tb = int(TB2[b])
                lk = int(LK2[b])   # even-parity chunks (row offset 0 vs G2W)
                if b + LOOKAHEAD < NBLK:
                    prep2(b + LOOKAHEAD)
                g2c = tiles2.pop(b)
                ohc = ohp.tile([P, tb * P], F16, tag="ohc")
                nc.vector.tensor_tensor(
                    out=_ap(ohc, 0, [[P, tb], [1, P]]),
                    in0=_ap(iota_sb, 0, [[0, tb], [1, P]]),
                    in1=_ap(dst2_sb, t0, [[1, tb], [0, P]]),
                    op=ALU.is_equal)
                bc = bcp.tile([P, tb * P], F16, tag="bc")
                nc.sync.dma_start(
                    out=bc[:],
                    in_=bass.AP(dstrow2.tensor, dstrow2.offset + t0 * P,
                                [[0, P], [1, tb * P]]))
                ohtc = ohtp.tile([P, tb * P], F16, tag="ohtc")
                nc.vector.tensor_tensor(
                    out=ohtc[:],
                    in0=_ap(iotac_sb, 0, [[0, tb], [0, P]]),
                    in1=bc[:],
                    op=ALU.is_equal)
                adps = psB.tile([P, tb], F32, tag="adps")
                for k in range(tb):
                    nc.tensor.matmul(out=adps[:, k:k + 1],
                                     lhsT=ohtc[:, k * P:(k + 1) * P],
                                     rhs=_ap(g2stage, b * G2W + G2R, [[1, 1]]),
                                     start=True, stop=True)
                ech = ep.tile([P, tb], F32, tag="ech2")
                if lk > 0:
                    nc.vector.tensor_tensor(
                        out=ech[:, 0:lk],
                        in0=_ap(g2c, 0, [[ROWW, lk]]),
                        in1=adps[:, 0:lk],
                        op=ALU.add)
                if tb > lk:
                    nc.vector.tensor_tensor(
                        out=ech[:, lk:tb],
                        in0=_ap(g2c, lk * ROWW + G2W, [[ROWW, tb - lk]]),
                        in1=adps[:, lk:tb],
                        op=ALU.add)
                lrch = ep.tile([P, tb], F32, tag="lrch2")
                nc.vector.scalar_tensor_tensor(out=lrch[:], in0=ech[:], scalar=0.2,
                                               in1=ech[:], op0=ALU.mult, op1=ALU.max)
                pch = ep.tile([P, tb], F32, tag="pch2")
                nc.scalar.activation(pch[:], lrch[:], ACTF.Exp, bias=zcol[:, 0:1])
                vc = vp.tile([P, tb * G2R], F16, tag="vc2")
                nc.vector.tensor_copy(out=_ap(vc, 0, [[G2R, tb]]), in_=pch[:])
                if lk > 0:
                    nc.vector.tensor_tensor(
                        out=_ap(vc, 1, [[G2R, lk], [1, NC]]),
                        in0=_ap(g2c, 1, [[ROWW, lk], [1, NC]]),
                        in1=_ap(pch, 0, [[1, lk], [0, NC]]),
                        op=ALU.mult)
                if tb > lk:
                    nc.vector.tensor_tensor(
                        out=_ap(vc, 1 + lk * G2R, [[G2R, tb - lk], [1, NC]]),
                        in0=_ap(g2c, lk * ROWW + G2W + 1, [[ROWW, tb - lk], [1, NC]]),
                        in1=_ap(pch, lk, [[1, tb - lk], [0, NC]]),
                        op=ALU.mult)
                psagg = psG.tile([P, G2R], F32, tag="psagg2")
                for k in range(tb):
                    nc.tensor.matmul(out=psagg[:], lhsT=ohc[:, k * P:(k + 1) * P],
                                     rhs=vc[:, k * G2R:(k + 1) * G2R],
                                     start=(k == 0), stop=(k == tb - 1))
                nc.scalar.activation(
                    _ap(aggst2, b * G2R, [[1, G2R]]), psagg[:],
                    ACTF.Copy, bias=0.0)
                t0 += tb

            # ---- batched layer-2 epilogue + log_softmax -----------------
            b1c = epi.tile([P, NBLK], F32, tag="b1c")
            nc.vector.tensor_tensor(
                out=b1c[:],
                in0=_ap(g2stage, 0, [[G2W, NBLK]]),
                in1=_ap(g2stage, G2R, [[G2W, NBLK]]),
                op=ALU.add)
            nc.vector.scalar_tensor_tensor(out=b1c[:], in0=b1c[:], scalar=0.2,
                                           in1=b1c[:], op0=ALU.mult, op1=ALU.max)
            pb = epi.tile([P, NBLK], F32, tag="pb")
            nc.scalar.activation(pb[:], b1c[:], ACTF.Exp, bias=zcol[:, 0:1])
            nc.vector.tensor_tensor(
                out=b1c[:], in0=_ap(aggst2, 0, [[G2R, NBLK]]),
                in1=pb[:], op=ALU.add)
            nc.vector.tensor_scalar_add(out=b1c[:], in0=b1c[:], scalar1=1e-16)
            nc.vector.reciprocal(out=b1c[:], in_=b1c[:])
            hn1 = epi.tile([P, NBLK * NC], F32, tag="hn1")
            nc.vector.tensor_tensor(
                out=hn1[:],
                in0=_ap(g2stage, 1, [[G2W, NBLK], [1, NC]]),
                in1=_ap(pb, 0, [[1, NBLK], [0, NC]]),
                op=ALU.mult)
            nc.vector.tensor_tensor(
                out=hn1[:], in0=_ap(aggst2, 1, [[G2R, NBLK], [1, NC]]),
                in1=hn1[:], op=ALU.add)
            nc.vector.tensor_tensor(
                out=hn1[:], in0=hn1[:],
                in1=_ap(b1c, 0, [[1, NBLK], [0, NC]]),
                op=ALU.mult)
            m2 = epi.tile([P, NBLK], F32, tag="m2")
            nc.vector.reduce_max(m2[:], _ap(hn1, 0, [[NC, NBLK], [1, NC]]),
                                 axis=AX.X)
            nc.vector.tensor_tensor(
                out=hn1[:], in0=hn1[:],
                in1=_ap(m2, 0, [[1, NBLK], [0, NC]]),
                op=ALU.subtract)
            hn2 = epi.tile([P, NBLK * NC], F16, tag="hn2")
            nc.scalar.activation(hn2[:], hn1[:], ACTF.Exp, bias=zcol[:, 0:1])
            nc.vector.reduce_sum(smst[:], _ap(hn2, 0, [[NC, NBLK], [1, NC]]),
                                 axis=AX.X)
            nc.scalar.activation(m2[:], smst[:], ACTF.Ln, bias=zcol[:, 0:1])
            nc.vector.tensor_tensor(
                out=hn2[:], in0=hn1[:],
                in1=_ap(m2, 0, [[1, NBLK], [0, NC]]),
                op=ALU.subtract)
            nc.sync.dma_start(out=out[:, :], in_=hn2[:])

    nc.compile()
    return nc


def _pos_map(NOB):
    def pos(tn):
        tn = np.asarray(tn)
        b = tn // P
        p = tn % P
        g = b // XC
        j = b % XC
        nb = np.minimum(XC, NOB - g * XC)
        return g * (XC * P) + p * nb + j
    return pos


def _prep(x, edge_src, edge_dst, W1, a1_src, a1_dst, b1, W2, a2_src, a2_dst, b2):
    N, F = x.shape
    H, C = a1_src.shape
    NC = W2.shape[1]
    HC = H * C
    NOWN = N // CORES
    NBLK = math.ceil(NOWN / P)
    OWNPAD = NBLK * P
    NFOR = N - NOWN
    FBLK = math.ceil(NFOR / P)
    NODE_BLKS = NBLK + FBLK
    NTAB = NODE_BLKS * P
    G1W = 8 + HC + 8

    pos = _pos_map(NODE_BLKS)

    core_of = edge_dst // NOWN
    per_core = []
    for c in range(CORES):
        m = core_of == c
        s, d = edge_src[m], edge_dst[m] - c * NOWN
        blk = d // P
        order = np.argsort(blk, kind='stable')
        per_core.append((s[order], d[order], blk[order]))

    # per-core per-block two-class edge lists by layer; global chunk counts.
    # Class membership + stored index value are parameterized: layer 1 splits
    # by table half (int16 range), layer 2 by pair parity (packed 2/row).
    def layer_split(pos_of_src, is_lo, lo_val, hi_val):
        """-> per-core lists of (low_idx_arrays, high_idx_arrays, d arrays)"""
        lows, highs = np.zeros((CORES, NBLK), np.int64), np.zeros((CORES, NBLK), np.int64)
        data = []
        for c in range(CORES):
            s, d, blk = per_core[c]
            ps = pos_of_src(c, s)
            bstart = np.concatenate([[0], np.cumsum(np.bincount(blk, minlength=NBLK))])
            rows = []
            for b in range(NBLK):
                lo, hi = bstart[b], bstart[b + 1]
                pb, db = ps[lo:hi], d[lo:hi]
                m = is_lo(pb)
                il = np.argsort(np.where(m, pb, 1 << 30), kind='stable')
                nl = int(m.sum())
                lowo, higho = il[:nl], il[nl:]
                rows.append((lo_val(pb[lowo]), db[lowo], hi_val(pb[higho]), db[higho]))
                lows[c, b], highs[c, b] = nl, len(pb) - nl
            data.append(rows)
        LK = np.maximum(0, np.ceil(lows.max(axis=0) / P)).astype(np.int64)
        HK = np.maximum(0, np.ceil(highs.max(axis=0) / P)).astype(np.int64)
        LK = np.maximum(LK, 1)      # at least one chunk so the block exists
        TB = LK + HK
        return data, LK, TB

    pos1 = {}
    for c in range(CORES):
        own_lo, own_hi = c * NOWN, (c + 1) * NOWN
        fore = np.concatenate([np.arange(0, own_lo), np.arange(own_hi, N)])
        tn_of = np.empty(N, np.int64)
        tn_of[own_lo:own_hi] = np.arange(NOWN)
        tn_of[fore] = OWNPAD + np.arange(NFOR)
        pos1[c] = (tn_of, fore)

    data1, LK1, TB1 = layer_split(
        lambda c, s: pos(pos1[c][0][s]),
        is_lo=lambda pb: pb < SPLIT,
        lo_val=lambda pb: pb, hi_val=lambda pb: pb - SPLIT)
    # layer 2: two g2 rows packed per 256B table row; index = pair, class =
    # parity (chunk-uniform so the in-row byte offset is fixed per chunk)
    NBLK2 = NBLK + (NBLK % 2)
    OWNPAD2 = P * NBLK2
    data2, LK2, TB2 = layer_split(
        lambda c, s: (s // NOWN) * OWNPAD2 + ((s % NOWN) % P) * NBLK2 + (s % NOWN) // P,
        is_lo=lambda pb: pb % 2 == 0,
        lo_val=lambda pb: pb // 2, hi_val=lambda pb: pb // 2)
    NT1, NT2 = int(TB1.sum()), int(TB2.sum())
    toff1 = np.concatenate([[0], np.cumsum(TB1)])
    toff2 = np.concatenate([[0], np.cumsum(TB2)])

    W1r = W1.reshape(F, H, C)
    wsrc = (W1r * a1_src[None]).sum(-1)
    wdst = (W1r * a1_dst[None]).sum(-1)
    w1aug = np.concatenate([wsrc, W1, wdst], axis=1).astype(np.float16)
    b1aug = np.zeros((1, G1W), np.float16)
    b1aug[0, 8:8 + HC] = b1.astype(np.float16)
    G2W = 1 + NC + 1 + 6
    w2aug = np.zeros((HC + 1, G2W), np.float16)
    w2aug[0:HC, 0] = (W2 @ a2_src[0]).astype(np.float16)
    w2aug[0:HC, 1:1 + NC] = W2.astype(np.float16)
    w2aug[0:HC, 1 + NC] = (W2 @ a2_dst[0]).astype(np.float16)
    w2aug[HC, 1:1 + NC] = b2.astype(np.float16)
    ones1 = np.ones((1, P), np.float16)
    iotaf = np.tile(np.arange(P, dtype=np.float16)[None, :], (P, 1))
    iotac = np.arange(P, dtype=np.float16)[:, None]
    ident = np.eye(P, dtype=np.float16)

    xT = np.ascontiguousarray(x.T)

    def pack_layer(rows, LK, TB, toff, NT):
        sg = np.zeros((P, NT * 8), np.int16)
        dstc = np.full((P, NT), -1.0, np.float16)
        for b in range(NBLK):
            base = int(toff[b])
            for (vals, dv, coff) in ((rows[b][0], rows[b][1], 0),
                                     (rows[b][2], rows[b][3], int(LK[b]))):
                n = len(vals)
                if n == 0:
                    continue
                i = np.arange(n)
                sg[i % 16, (base + coff) * 8 + i // 16] = vals.astype(np.int16)
                dstc[i % P, base + coff + i // P] = (dv % P).astype(np.float16)
        for rep in range(1, 8):
            sg[rep * 16:(rep + 1) * 16, :] = sg[0:16, :]
        return sg, dstc

    in_maps = []
    for c in range(CORES):
        own_lo, own_hi = c * NOWN, (c + 1) * NOWN
        fore = pos1[c][1]
        xTp = np.zeros((F, NTAB), np.float16)
        xTp[:, 0:NOWN] = xT[:, own_lo:own_hi].astype(np.float16)
        xTp[:, OWNPAD:OWNPAD + NFOR] = xT[:, fore].astype(np.float16)

        sg1, dstc1 = pack_layer(data1[c], LK1, TB1, toff1, NT1)
        sg2, dstc2 = pack_layer(data2[c], LK2, TB2, toff2, NT2)
        in_maps.append({
            "xT": xTp, "w1aug": w1aug, "b1aug": b1aug, "w2aug": w2aug,
            "ones1": ones1, "iotaf": iotaf, "iotac": iotac, "ident": ident,
            "sg1": sg1, "sg2": sg2, "dstc1": dstc1, "dstc2": dstc2,
            "dstrow1": dstc1.T.reshape(1, NT1 * P),
            "dstrow2": dstc2.T.reshape(1, NT2 * P),
        })
    meta = dict(N=N, F=F, H=H, C=C, NC=NC, TB1=TB1, TB2=TB2, LK1=LK1, LK2=LK2,
                NTAB=NTAB, NBLK=NBLK, NODE_BLKS=NODE_BLKS, NOWN=NOWN)
    return in_maps, meta


_CACHED = {}


def run(inputs, eshift=-4.0, trace=False, tmpdir=None):
    in_maps, meta = _prep(**inputs)
    has_b1 = bool(np.any(np.asarray(inputs["b1"])))
    key = (meta["N"], meta["F"], meta["NC"], tuple(meta["TB1"]), tuple(meta["TB2"]),
           has_b1)
    if key not in _CACHED:
        _CACHED[key] = _build_program(meta["N"], meta["F"], meta["H"], meta["C"],
                                      meta["NC"], meta["TB1"], meta["TB2"],
                                      meta["LK1"], meta["LK2"], meta["NTAB"],
                                      meta["NBLK"], meta["NODE_BLKS"], eshift,
                                      has_b1)
    nc = _CACHED[key]
    kw = {"tmpdir": tmpdir} if tmpdir else {}
    res = bass_utils.run_bass_kernel_spmd(nc, in_maps,
                                          core_ids=list(range(CORES)),
                                          trace=trace, **kw)
    NOWN, NBLK, NC = meta["NOWN"], meta["NBLK"], meta["NC"]
    outs = []
    for c in range(CORES):
        o = res.results[c]["out"]
        full = o.reshape(P, NBLK, NC).transpose(1, 0, 2).reshape(NBLK * P, NC)
        outs.append(full[:NOWN])
    full = np.concatenate(outs, axis=0).astype(np.float32)
    return full, res


def kernel(**inputs):
    full, _ = run(inputs)
    return full

